# revision 1
# baseline (speedup 1.0000x reference)
"""KAN layer (per-edge tiny MLPs) Trainium2 kernel.

Math (per batch b, output o, input i; H=32 hidden):
  h1 = leaky(x[b,i]*W1[o,i,:] + b1[o,i,:])
  z2 = W2[o,i] @ h1 + b2[o,i]           (per-edge [H,H] matmul)
  h2 = leaky(z2)
  edge = W3[o,i]·h2 + b3[o,i]
  out[b,o] = sum_i (bias_w[o,i]*leaky(x[b,i]) + layer_w[o,i]*edge)

Mapping (8 cores, O sharded, 8 output rows per core):
  - x replicated 32x on host -> ACT computes h1 = Lrelu(W1[p]*xrep + b1[p])
    in one pass per (o, i-group of 4), layout [128=(4i x 32h), B].
  - PE (float32r): block-diagonal W2^T [128,128] per (o,g) -> z2 in PSUM;
    folded contractions: st4 = c2 * (layer_w*W3) on h2-ish, st4b = w~2 on h1
    ... actually h2 here is the true leaky, so st4 = layer_w*W3 directly.
  - z2 evac: ACT Lrelu(z2 + b2[p]) or DVE 2-pass leaky (load balance split).
  - All output contractions accumulate into one [8, B] PSUM region:
    st4[og] [128,8] (col o = layer_w*W3 stack), st5 [65,8] carries
    bias_w·leaky(x) + all constants.
"""
import sys

sys.path.insert(0, "/opt/trn_rl_repo")

import numpy as np

_B, _I, _O, _H = 1024, 64, 64, 32
_NCORES = 8
_OLOC = _O // _NCORES  # 8 output nodes per core
_ALPHA = 0.01
_NHALF = 512

# (o,g) blocks whose z2-evac runs on DVE (2-pass leaky) instead of ACT:
# DVE is ~2.4x the per-element cost of ACT here, but ACT also carries all of
# h1 generation, so ~60% of evacs go to DVE to balance the two engines.
def _on_dve(og):
    return og % 5 < 3

_CACHE = {}


def _build_bass():
    import concourse.bacc as bacc
    import concourse.mybir as mybir
    from concourse.tile import TileContext

    f32 = mybir.dt.float32
    f32r = mybir.dt.float32r
    AF = mybir.ActivationFunctionType
    ALU = mybir.AluOpType

    nc = bacc.Bacc("TRN2", target_bir_lowering=False, debug=False)

    xrep_d = nc.declare_dram_parameter("xrep", [2048, _B], f32, isOutput=False)
    xt65_d = nc.declare_dram_parameter("xt65", [65, _B], f32, isOutput=False)
    w1col_d = nc.declare_dram_parameter("w1col", [128, 128], f32, isOutput=False)
    b1col_d = nc.declare_dram_parameter("b1col", [128, 128], f32, isOutput=False)
    b2col_d = nc.declare_dram_parameter("b2col", [128, 128], f32, isOutput=False)
    w2blk_d = nc.declare_dram_parameter("w2blk", [128, 128, 128], f32r, isOutput=False)
    st4_d = nc.declare_dram_parameter("st4", [128, 128 * 8], f32r, isOutput=False)
    st5_d = nc.declare_dram_parameter("st5", [65, 8], f32r, isOutput=False)
    out_d = nc.declare_dram_parameter("out", [8, _B], f32, isOutput=True)

    with TileContext(nc) as tc:
        with tc.tile_pool(name="consts", bufs=1) as cpool, \
             tc.tile_pool(name="w2", bufs=2) as w2pool, \
             tc.tile_pool(name="h1", bufs=5) as h1pool, \
             tc.tile_pool(name="h2", bufs=5) as h2pool, \
             tc.tile_pool(name="a01", bufs=4) as a01pool, \
             tc.tile_pool(name="zps", bufs=3, space="PSUM") as zpool, \
             tc.tile_pool(name="ops", bufs=1, space="PSUM") as opool:

            xrep_t = cpool.tile([128, 16 * _B], f32)
            nc.sync.dma_start(
                out=xrep_t[:].rearrange("p (g n) -> p g n", g=16),
                in_=xrep_d[:].rearrange("(g p) n -> p g n", p=128),
            )
            xt65_t = cpool.tile([65, _B], f32)
            nc.sync.dma_start(out=xt65_t[:], in_=xt65_d[:])
            w1col_t = cpool.tile([128, 128], f32)
            nc.sync.dma_start(out=w1col_t[:], in_=w1col_d[:])
            b1col_t = cpool.tile([128, 128], f32)
            nc.sync.dma_start(out=b1col_t[:], in_=b1col_d[:])
            b2col_t = cpool.tile([128, 128], f32)
            nc.sync.dma_start(out=b2col_t[:], in_=b2col_d[:])
            st4_t = cpool.tile([128, 128 * 8], f32r)
            nc.sync.dma_start(out=st4_t[:], in_=st4_d[:])
            st5_t = cpool.tile([65, 8], f32r)
            nc.sync.dma_start(out=st5_t[:], in_=st5_d[:])

            lxT_t = cpool.tile([65, _B], f32r)
            nc.scalar.activation(lxT_t[:], xt65_t[:], AF.Lrelu,
                                 bias=0.0, scale=1.0, alpha=_ALPHA)

            outp = opool.tile([8, _B], f32)
            # MM5 first: seeds the accumulator (start=True per half/bank)
            for half in range(2):
                sl = slice(half * _NHALF, (half + 1) * _NHALF)
                nc.tensor.matmul(out=outp[:, sl], lhsT=st5_t[:], rhs=lxT_t[:, sl],
                                 start=True, stop=False, skip_group_check=True)

            def emit_mm4(h2_prev, og_prev, last):
                for half in range(2):
                    sl = slice(half * _NHALF, (half + 1) * _NHALF)
                    nc.tensor.matmul(out=outp[:, sl],
                                     lhsT=st4_t[:, og_prev * 8:(og_prev + 1) * 8],
                                     rhs=h2_prev[:, sl], start=False, stop=last,
                                     skip_group_check=True)

            pending = None  # (h2, og) one block behind, so PE never waits on evac
            for o in range(_OLOC):
                w2_t = w2pool.tile([128, 16 * 128], f32r)
                nc.sync.dma_start(
                    out=w2_t[:].rearrange("p (g m) -> p g m", g=16),
                    in_=w2blk_d[o * 16:(o + 1) * 16].rearrange("g p m -> p g m"),
                )
                for g in range(16):
                    og = o * 16 + g
                    h1 = h1pool.tile([128, _B], f32r)
                    nc.scalar.activation(
                        h1[:], xrep_t[:, g * _B:(g + 1) * _B], AF.Lrelu,
                        bias=b1col_t[:, og:og + 1], scale=w1col_t[:, og:og + 1],
                        alpha=_ALPHA)
                    z2 = zpool.tile([128, _B], f32)
                    for half in range(2):
                        sl = slice(half * _NHALF, (half + 1) * _NHALF)
                        nc.tensor.matmul(out=z2[:, sl],
                                         lhsT=w2_t[:, g * 128:(g + 1) * 128],
                                         rhs=h1[:, sl], start=True, stop=True)
                    h2 = h2pool.tile([128, _B], f32r)
                    if _on_dve(og):
                        a01 = a01pool.tile([128, _B], f32)
                        nc.vector.tensor_scalar(
                            out=a01[:], in0=z2[:], scalar1=b2col_t[:, og:og + 1],
                            scalar2=_ALPHA, op0=ALU.add, op1=ALU.mult)
                        nc.vector.scalar_tensor_tensor(
                            out=h2[:], in0=z2[:], scalar=b2col_t[:, og:og + 1],
                            in1=a01[:], op0=ALU.add, op1=ALU.max)
                    else:
                        nc.scalar.activation(h2[:], z2[:], AF.Lrelu,
                                             bias=b2col_t[:, og:og + 1],
                                             scale=1.0, alpha=_ALPHA)
                    if pending is not None:
                        emit_mm4(*pending, last=False)
                    pending = (h2, og)
            emit_mm4(*pending, last=True)

            outs = cpool.tile([8, _B], f32)
            nc.vector.tensor_copy(outs[:], outp[:])
            nc.sync.dma_start(out=out_d[:], in_=outs[:])

    nc.finalize()
    return nc


def _prepare_inputs(x, W1, b1, W2, b2, W3, b3, layer_w, bias_w):
    c1 = (1.0 + _ALPHA) / 2.0
    f = np.float32
    x = np.asarray(x, f)
    xT = np.ascontiguousarray(x.T)                      # [I, B]
    xrep = np.repeat(xT, _H, axis=0)                    # [2048, B]
    xt65 = np.concatenate([xT, np.ones((1, _B), f)], 0)  # [65, B]

    v = (np.asarray(layer_w, f)[:, :, None] * np.asarray(W3, f))  # [O,I,H]
    w2f = np.asarray(W2, f)

    in_maps = []
    for c in range(_NCORES):
        sl = slice(c * _OLOC, (c + 1) * _OLOC)
        W1c, b1c, b2c = W1[sl], b1[sl], b2[sl]          # [8,64,H]
        W2c = w2f[sl]                                   # [8,64,H,H]
        vc = v[sl]
        lwc, bwc, b3c = layer_w[sl], bias_w[sl], b3[sl]

        # [o, g, j, h] -> partition 32j+h, col o*16+g
        def cols(a):  # a [8, 64, 32] -> [128, 128]
            a = np.asarray(a, f).reshape(_OLOC, 16, 4, _H)
            return np.ascontiguousarray(
                a.transpose(2, 3, 0, 1).reshape(128, 128))

        w1col = cols(W1c)
        b1col = cols(b1c)
        b2col = cols(b2c)

        # block-diagonal lhsT: blk[og][32j+h, 32j+k] = W2[o,4g+j,k,h]
        W2t = W2c.transpose(0, 1, 3, 2).reshape(_OLOC, 16, 4, _H, _H)
        w2blk = np.zeros((_OLOC, 16, 128, 128), f)
        for j in range(4):
            w2blk[:, :, 32 * j:32 * j + 32, 32 * j:32 * j + 32] = W2t[:, :, j]
        w2blk = w2blk.reshape(128, 128, 128)

        # st4[og][32j+k, o] = v[o,4g+j,k] ; st4b[og][32j+h, o] = wt2[o,4g+j,h]
        def stack8b(a):
            a = np.asarray(a, f).reshape(_OLOC, 16, 4 * _H)
            out = np.zeros((128, _OLOC * 16, _OLOC), f)
            for o in range(_OLOC):
                for g in range(16):
                    out[:, o * 16 + g, o] = a[o, g]
            return np.ascontiguousarray(out.reshape(128, 128 * _OLOC))

        st4 = stack8b(vc)

        st5 = np.zeros((65, _OLOC), f)
        st5[:_I, :] = np.asarray(bwc, f).T              # bias_w[o,i] at row i
        const = (np.asarray(lwc, f) * np.asarray(b3c, f)).sum(1)
        st5[_I, :] = const

        in_maps.append({
            "xrep": xrep, "xt65": xt65,
            "w1col": w1col, "b1col": b1col, "b2col": b2col,
            "w2blk": w2blk, "st4": st4, "st5": st5,
        })
    return in_maps


def kernel(x, W1, b1, W2, b2, W3, b3, layer_w, bias_w):
    from concourse.bass_utils import run_bass_kernel_spmd

    if "nc" not in _CACHE:
        _CACHE["nc"] = _build_bass()
    nc = _CACHE["nc"]

    in_maps = _prepare_inputs(x, W1, b1, W2, b2, W3, b3, layer_w, bias_w)
    res = run_bass_kernel_spmd(nc, in_maps, list(range(_NCORES))).results

    out = np.empty((_B, _O), np.float32)
    for c in range(_NCORES):
        out[:, c * _OLOC:(c + 1) * _OLOC] = res[c]["out"].T
    return out


if __name__ == "__main__":
    # quick self-check against a numpy reference
    rng = np.random.default_rng(0)
    f = np.float32
    inputs = {
        "x": rng.standard_normal((_B, _I), f),
        "W1": rng.uniform(-1, 1, (_O, _I, _H)).astype(f),
        "b1": rng.uniform(-1, 1, (_O, _I, _H)).astype(f),
        "W2": rng.uniform(-0.2, 0.2, (_O, _I, _H, _H)).astype(f),
        "b2": rng.uniform(-0.2, 0.2, (_O, _I, _H)).astype(f),
        "W3": rng.uniform(-0.2, 0.2, (_O, _I, _H)).astype(f),
        "b3": rng.uniform(-0.2, 0.2, (_O, _I)).astype(f),
        "layer_w": np.ones((_O, _I), f),
        "bias_w": rng.uniform(-0.1, 0.1, (_O, _I)).astype(f),
    }

    def leaky(a):
        return np.where(a >= 0, a, _ALPHA * a)

    def ref(x, W1, b1, W2, b2, W3, b3, layer_w, bias_w):
        h1 = leaky(x[:, None, :, None] * W1 + b1)
        h2 = leaky(np.einsum("boih,oikh->boik", h1, W2) + b2)
        edge = np.einsum("boih,oih->boi", h2, W3) + b3
        edge = bias_w * leaky(x)[:, None, :] + layer_w * edge
        return edge.sum(axis=2)

    expected = ref(**{k: np.asarray(val, np.float64) for k, val in inputs.items()})
    actual = kernel(**inputs)
    err = np.abs(actual - expected).max() / np.abs(expected).max()
    print("rel err:", err)



# revision 2
# speedup vs baseline: 1.7631x; 1.7631x over previous
"""KAN layer (per-edge tiny MLPs) Trainium2 kernel.

Math (per batch b, output o, input i; H=32 hidden):
  h1 = leaky(x[b,i]*W1[o,i,:] + b1[o,i,:])
  z2 = W2[o,i] @ h1 + b2[o,i]           (per-edge [H,H] matmul)
  h2 = leaky(z2)
  edge = W3[o,i]·h2 + b3[o,i]
  out[b,o] = sum_i (bias_w[o,i]*leaky(x[b,i]) + layer_w[o,i]*edge)

Mapping (8 cores, O sharded, 8 output rows per core), bf16 datapath:
  - x replicated 32x on host (bf16) -> h1 = max(t, 0.01*t), t = W1*x+b1,
    computed per (o, i-group of 4) in layout [128=(4i x 32h), B].
    h1 producers balanced across DVE (tensor_scalar 4x-mode x2 + tensor_tensor
    2x-mode) and Pool/GPSIMD (tensor_scalar + scalar_tensor_tensor), with ACT
    handling the z2-evacs (1-op Lrelu w/ per-partition scale+bias).
  - PE (bf16): block-diagonal W2^T [128,128] per (o,g) -> z2 in PSUM;
    st4 = layer_w*W3 columns contract h2 into one [8, B] PSUM accumulator;
    st5 [65,8] carries bias_w·leaky(x) + constant terms via MM5.
  - All engines balanced ~122us; PE ~109us (cost = out-cols * 0.417ns/row,
    bf16 = 1 cycle/row).
"""
import sys

sys.path.insert(0, "/opt/trn_rl_repo")

import numpy as np
import ml_dtypes

_B, _I, _O, _H = 1024, 64, 64, 32
_NCORES = 8
_OLOC = _O // _NCORES  # 8 output nodes per core
_ALPHA = 0.01
_NHALF = 512
_BF16 = ml_dtypes.bfloat16


# --- engine assignment per (o,g) block: balance ACT / DVE / Pool ---
def _h1_dve(og):  # else Pool
    return og % 3 != 0


def _evac_pool(og):  # else ACT
    return og % 5 == 2


_CACHE = {}


def _build_bass():
    import concourse.bacc as bacc
    import concourse.mybir as mybir
    from concourse.tile import TileContext

    f32 = mybir.dt.float32
    bf16 = mybir.dt.bfloat16
    AF = mybir.ActivationFunctionType
    ALU = mybir.AluOpType

    nc = bacc.Bacc("TRN2", target_bir_lowering=False, debug=False)

    xrep_d = nc.declare_dram_parameter("xrepb", [128, 16 * _B], bf16, isOutput=False)
    xt65_d = nc.declare_dram_parameter("xt65", [65, _B], f32, isOutput=False)
    w1col_d = nc.declare_dram_parameter("w1col", [128, 128], f32, isOutput=False)
    b1col_d = nc.declare_dram_parameter("b1col", [128, 128], f32, isOutput=False)
    b2col_d = nc.declare_dram_parameter("b2col", [128, 128], f32, isOutput=False)
    # [p, og, m] layout so the per-o DMA slice is contiguous per partition
    w2blk_d = nc.declare_dram_parameter("w2blkb", [128, 128, 128], bf16, isOutput=False)
    st4_d = nc.declare_dram_parameter("st4b", [128, 128 * 8], bf16, isOutput=False)
    st5_d = nc.declare_dram_parameter("st5b", [65, 8], bf16, isOutput=False)
    out_d = nc.declare_dram_parameter("out", [8, _B], f32, isOutput=True)

    with TileContext(nc) as tc:
        with tc.tile_pool(name="consts", bufs=1) as cpool, \
             tc.tile_pool(name="w2", bufs=2) as w2pool, \
             tc.tile_pool(name="h1", bufs=6) as h1pool, \
             tc.tile_pool(name="h2", bufs=4) as h2pool, \
             tc.tile_pool(name="tmp", bufs=6) as tmppool, \
             tc.tile_pool(name="zps", bufs=3, space="PSUM") as zpool, \
             tc.tile_pool(name="ops", bufs=1, space="PSUM") as opool:

            # xrep: 16 per-g slice DMAs so compute starts after the first slice
            xrep_t = cpool.tile([128, 16 * _B], bf16)
            nc.sync.dma_start(out=xrep_t[:, 0:_B], in_=xrep_d[:, 0:_B])

            xt65_t = cpool.tile([65, _B], f32)
            nc.sync.dma_start(out=xt65_t[:], in_=xt65_d[:])
            w1col_t = cpool.tile([128, 128], f32)
            nc.sync.dma_start(out=w1col_t[:], in_=w1col_d[:])
            b1col_t = cpool.tile([128, 128], f32)
            nc.sync.dma_start(out=b1col_t[:], in_=b1col_d[:])
            b2col_t = cpool.tile([128, 128], f32)
            nc.sync.dma_start(out=b2col_t[:], in_=b2col_d[:])
            st4_t = cpool.tile([128, 128 * 8], bf16)
            nc.sync.dma_start(out=st4_t[:], in_=st4_d[:])
            st5_t = cpool.tile([65, 8], bf16)
            nc.sync.dma_start(out=st5_t[:], in_=st5_d[:])

            lxT_t = cpool.tile([65, _B], bf16)
            nc.scalar.activation(lxT_t[:], xt65_t[:], AF.Lrelu,
                                 bias=0.0, scale=1.0, alpha=_ALPHA)

            outp = opool.tile([8, _B], f32)
            # MM5 first: seeds the accumulator (start=True per half/bank)
            for half in range(2):
                sl = slice(half * _NHALF, (half + 1) * _NHALF)
                nc.tensor.matmul(out=outp[:, sl], lhsT=st5_t[:], rhs=lxT_t[:, sl],
                                 start=True, stop=False, skip_group_check=True)

            def emit_mm4(h2_prev, og_prev, last):
                for half in range(2):
                    sl = slice(half * _NHALF, (half + 1) * _NHALF)
                    nc.tensor.matmul(out=outp[:, sl],
                                     lhsT=st4_t[:, og_prev * 8:(og_prev + 1) * 8],
                                     rhs=h2_prev[:, sl], start=False, stop=last,
                                     skip_group_check=True)

            pending = None  # (h2, og) one block behind, so PE never waits on evac
            for o in range(_OLOC):
                w2_t = w2pool.tile([128, 16 * 128], bf16)
                nc.sync.dma_start(
                    out=w2_t[:],
                    in_=w2blk_d[:, o * 16:(o + 1) * 16].rearrange("p g m -> p (g m)"),
                )
                for g in range(16):
                    og = o * 16 + g
                    if o == 0 and g < 15:
                        # prefetch next xrep slice during the first o pass
                        gn = g + 1
                        nc.sync.dma_start(out=xrep_t[:, gn * _B:(gn + 1) * _B],
                                          in_=xrep_d[:, gn * _B:(gn + 1) * _B])
                    xg = xrep_t[:, g * _B:(g + 1) * _B]
                    h1 = h1pool.tile([128, _B], bf16)
                    if _h1_dve(og):
                        t = tmppool.tile([128, _B], bf16)
                        a = tmppool.tile([128, _B], bf16)
                        nc.vector.tensor_scalar(
                            out=t[:], in0=xg, scalar1=w1col_t[:, og:og + 1],
                            scalar2=b1col_t[:, og:og + 1], op0=ALU.mult, op1=ALU.add)
                        nc.vector.tensor_scalar(
                            out=a[:], in0=t[:], scalar1=_ALPHA, scalar2=None,
                            op0=ALU.mult)
                        nc.vector.tensor_tensor(
                            out=h1[:], in0=t[:], in1=a[:], op=ALU.max)
                    else:
                        t = tmppool.tile([128, _B], bf16)
                        nc.gpsimd.tensor_scalar(
                            out=t[:], in0=xg, scalar1=w1col_t[:, og:og + 1],
                            scalar2=b1col_t[:, og:og + 1], op0=ALU.mult, op1=ALU.add)
                        nc.gpsimd.scalar_tensor_tensor(
                            out=h1[:], in0=t[:], scalar=_ALPHA, in1=t[:],
                            op0=ALU.mult, op1=ALU.max)
                    z2 = zpool.tile([128, _B], f32)
                    for half in range(2):
                        sl = slice(half * _NHALF, (half + 1) * _NHALF)
                        nc.tensor.matmul(out=z2[:, sl],
                                         lhsT=w2_t[:, g * 128:(g + 1) * 128],
                                         rhs=h1[:, sl], start=True, stop=True)
                    h2 = h2pool.tile([128, _B], bf16)
                    if _evac_pool(og):
                        t2 = tmppool.tile([128, _B], bf16)
                        nc.gpsimd.tensor_scalar(
                            out=t2[:], in0=z2[:], scalar1=b2col_t[:, og:og + 1],
                            scalar2=None, op0=ALU.add)
                        nc.gpsimd.scalar_tensor_tensor(
                            out=h2[:], in0=t2[:], scalar=_ALPHA, in1=t2[:],
                            op0=ALU.mult, op1=ALU.max)
                    else:
                        nc.scalar.activation(h2[:], z2[:], AF.Lrelu,
                                             bias=b2col_t[:, og:og + 1],
                                             scale=1.0, alpha=_ALPHA)
                    if pending is not None:
                        emit_mm4(*pending, last=False)
                    pending = (h2, og)
            emit_mm4(*pending, last=True)

            outs = cpool.tile([8, _B], f32)
            nc.gpsimd.tensor_copy(outs[:], outp[:])
            nc.sync.dma_start(out=out_d[:], in_=outs[:])

    nc.finalize()
    return nc


def _prepare_inputs(x, W1, b1, W2, b2, W3, b3, layer_w, bias_w):
    f = np.float32
    x = np.asarray(x, f)
    xT = np.ascontiguousarray(x.T)                      # [I, B]
    # xrepb[32j+h, g*B+b] = x[4g+j, b]
    xq = xT.reshape(16, 4, _B).transpose(1, 0, 2)       # [j, g, b]
    xrepb = np.ascontiguousarray(
        np.repeat(xq, _H, axis=0).reshape(128, 16 * _B)).astype(_BF16)
    xt65 = np.concatenate([xT, np.ones((1, _B), f)], 0)  # [65, B]

    v = (np.asarray(layer_w, f)[:, :, None] * np.asarray(W3, f))  # [O,I,H]
    w2f = np.asarray(W2, f)

    in_maps = []
    for c in range(_NCORES):
        sl = slice(c * _OLOC, (c + 1) * _OLOC)
        W1c, b1c, b2c = W1[sl], b1[sl], b2[sl]          # [8,64,H]
        W2c = w2f[sl]                                   # [8,64,H,H]
        vc = v[sl]
        lwc, bwc, b3c = layer_w[sl], bias_w[sl], b3[sl]

        # [o, g, j, h] -> partition 32j+h, col o*16+g
        def cols(a):  # a [8, 64, 32] -> [128, 128]
            a = np.asarray(a, f).reshape(_OLOC, 16, 4, _H)
            return np.ascontiguousarray(
                a.transpose(2, 3, 0, 1).reshape(128, 128))

        w1col = cols(W1c)
        b1col = cols(b1c)
        b2col = cols(b2c)

        # block-diagonal lhsT: blk[og][32j+h, 32j+k] = W2[o,4g+j,k,h]
        W2t = W2c.transpose(0, 1, 3, 2).reshape(_OLOC, 16, 4, _H, _H)
        w2blk = np.zeros((_OLOC, 16, 128, 128), f)
        for j in range(4):
            w2blk[:, :, 32 * j:32 * j + 32, 32 * j:32 * j + 32] = W2t[:, :, j]
        # -> [p, og, m] layout, bf16
        w2blkb = np.ascontiguousarray(
            w2blk.reshape(128, 128, 128).transpose(1, 0, 2)).astype(_BF16)

        # st4[og][32j+k, o] = v[o,4g+j,k]
        def stack8b(a):
            a = np.asarray(a, f).reshape(_OLOC, 16, 4 * _H)
            out = np.zeros((128, _OLOC * 16, _OLOC), f)
            for o in range(_OLOC):
                for g in range(16):
                    out[:, o * 16 + g, o] = a[o, g]
            return np.ascontiguousarray(out.reshape(128, 128 * _OLOC))

        st4b = stack8b(vc).astype(_BF16)

        st5 = np.zeros((65, _OLOC), f)
        st5[:_I, :] = np.asarray(bwc, f).T              # bias_w[o,i] at row i
        const = (np.asarray(lwc, f) * np.asarray(b3c, f)).sum(1)
        st5[_I, :] = const
        st5b = st5.astype(_BF16)

        in_maps.append({
            "xrepb": xrepb, "xt65": xt65,
            "w1col": w1col, "b1col": b1col, "b2col": b2col,
            "w2blkb": w2blkb, "st4b": st4b, "st5b": st5b,
        })
    return in_maps


def kernel(x, W1, b1, W2, b2, W3, b3, layer_w, bias_w):
    from concourse.bass_utils import run_bass_kernel_spmd

    if "nc" not in _CACHE:
        _CACHE["nc"] = _build_bass()
    nc = _CACHE["nc"]

    in_maps = _prepare_inputs(x, W1, b1, W2, b2, W3, b3, layer_w, bias_w)
    res = run_bass_kernel_spmd(nc, in_maps, list(range(_NCORES))).results

    out = np.empty((_B, _O), np.float32)
    for c in range(_NCORES):
        out[:, c * _OLOC:(c + 1) * _OLOC] = res[c]["out"].T
    return out


if __name__ == "__main__":
    # quick self-check against a numpy reference
    rng = np.random.default_rng(0)
    f = np.float32
    inputs = {
        "x": rng.standard_normal((_B, _I), f),
        "W1": rng.uniform(-1, 1, (_O, _I, _H)).astype(f),
        "b1": rng.uniform(-1, 1, (_O, _I, _H)).astype(f),
        "W2": rng.uniform(-0.2, 0.2, (_O, _I, _H, _H)).astype(f),
        "b2": rng.uniform(-0.2, 0.2, (_O, _I, _H)).astype(f),
        "W3": rng.uniform(-0.2, 0.2, (_O, _I, _H)).astype(f),
        "b3": rng.uniform(-0.2, 0.2, (_O, _I)).astype(f),
        "layer_w": np.ones((_O, _I), f),
        "bias_w": rng.uniform(-0.1, 0.1, (_O, _I)).astype(f),
    }

    def leaky(a):
        return np.where(a >= 0, a, _ALPHA * a)

    def ref(x, W1, b1, W2, b2, W3, b3, layer_w, bias_w):
        h1 = leaky(x[:, None, :, None] * W1 + b1)
        h2 = leaky(np.einsum("boih,oikh->boik", h1, W2) + b2)
        edge = np.einsum("boih,oih->boi", h2, W3) + b3
        edge = bias_w * leaky(x)[:, None, :] + layer_w * edge
        return edge.sum(axis=2)

    expected = ref(**{k: np.asarray(val, np.float64) for k, val in inputs.items()})
    actual = kernel(**inputs)
    err = np.abs(actual - expected).max() / np.abs(expected).max()
    print("rel err:", err)
